# revision 14
# baseline (speedup 1.0000x reference)
"""Trainium2 Bass kernel for nn_LowRankRotatedSpaceIntervention.

Reference computation (B=8192, D=4096, r=512, k=128):
    sel  = subspaces[0]                  # shared index set (fast path)
    diff = (source - base) @ W           # [B, r]
    out  = base + diff[:, sel] @ W[:, sel].T

Only the selected k=128 columns of W matter:
    out = base + ((source - base) @ W_sel) @ W_sel.T,  W_sel = W[:, sel]

The problem is HBM-bound (per-core DMA ceiling ~320 GB/s while busy, PE
needs only ~45us of matmul at the throttled pstate), so the kernel is
organized purely around minimizing HBM bytes and keeping the DMA engines
continuously busy:

  * base/source are packed on the host into a TRANSPOSED chunk-major
    16-bit layout so the device needs no transposes: the contraction dim
    (d) is already on partitions.  All FLOPs of the reference graph
    (sub, both matmuls, final add) stay on device; the host only does
    dtype conversion, layout packing and the W-column gather.
  * device I/O is bf16 (base, out) and fp8-e4m3 (source; its rounding
    error only enters through the rank-k correction, contributing
    ~3e-3 relative).  Measured end-to-end rel err: 5.9e-3 (budget 2e-2).
  * w2 = W_sel.T is derived on-device by PE-transposing w1 during the
    DMA ramp instead of loading it (saves 1 MiB of weight traffic).
  * batch is cut into tiles of [256,256,256,128,128] rows: big tiles in
    the steady state (DMA efficiency), small tiles at the end so the
    serial mm2->add->store drain of the last tile is short.
  * emission is software-pipelined: tile t+1's loads+subs are emitted
    before tile t's mm/add phase so the in-order DVE runs subs(t+1)
    ahead of adds(t); the largest loads are armed first so the DMA
    queues never run dry during the ramp.

Per-core dataflow (BS=1024 rows/core, NCH=32 d-chunks of 128):
    sT/bT dram [128, 32*BS]: [p, off_t + j*TB_t + b] = x[bat_t + b, 128j + p]
    per batch tile t:
      diffT = bf16(sT) - bT                 (scalar convert + DVE sub)
      T^T[k,TB]  = sum_j w1_j.T @ diffT_j   (32 matmuls, psum f32)
      tt = bf16(T^T)                        (scalar engine copy)
      per chunk j: corrT_j = w2_j.T @ tt    (matmul, 4-chunk psum tiles)
      outT_j = bT_j + corrT_j               (DVE add, 4 chunks at a time)
      store outT in groups of 8 chunks
"""

import numpy as np
import ml_dtypes

import concourse.bass as bass
import concourse.tile as tile
from concourse import bacc, masks, mybir
from concourse.bass_utils import run_bass_kernel_spmd

N_CORES = 8
B_FULL = 8192
D = 4096
K = 128
BS = B_FULL // N_CORES        # 1024 rows per core
TBS = (256, 256, 256, 128, 128)  # batch tile sizes (sum = BS)
NT = len(TBS)
NCH = D // 128                # 32 contraction / output chunks
GCH = 8                       # chunks per store group
G = NCH // GCH                # 4 store groups per tile
PCH = 4                       # mm2 chunks per psum tile / DVE add
TBMAX = max(TBS)

assert sum(TBS) == BS

F32 = mybir.dt.float32
BF16 = mybir.dt.bfloat16
FP8 = mybir.dt.float8e4


def _build(src_dtype="fp8"):
    nc = bacc.Bacc("TRN2", target_bir_lowering=False, debug=False)

    s_dt = BF16 if src_dtype == "bf16" else FP8
    sT_d = nc.dram_tensor("sT", [128, NCH * BS], s_dt, kind="ExternalInput").ap()
    bT_d = nc.dram_tensor("bT", [128, NCH * BS], BF16, kind="ExternalInput").ap()
    # w1: chunk-major W_sel: w1[p, 128*j + kk] = W_sel[128*j + p, kk]
    # (w2 = W_sel.T is derived on-device by PE-transposing w1)
    w1_d = nc.dram_tensor("w1", [128, D], BF16, kind="ExternalInput").ap()
    out_d = nc.dram_tensor("out", [128, NCH * BS], BF16, kind="ExternalOutput").ap()

    in_place_sub = s_dt == BF16
    offs = [NCH * sum(TBS[:t]) for t in range(NT + 1)]  # dram col offsets

    with tile.TileContext(nc) as tc:
        with (
            tc.tile_pool(name="wpool", bufs=1) as wpool,
            tc.tile_pool(name="spool", bufs=4) as spool,
            tc.tile_pool(name="bpool", bufs=4) as bpool,
            tc.tile_pool(name="dpool", bufs=2) as dpool,
            tc.tile_pool(name="ttpool", bufs=2) as ttpool,
            tc.tile_pool(name="opool", bufs=4) as opool,
            tc.tile_pool(name="pT", bufs=2, space="PSUM") as pTpool,
            tc.tile_pool(name="p2", bufs=2, space="PSUM") as p2pool,
            tc.tile_pool(name="ptr", bufs=2, space="PSUM") as ptrpool,
        ):
            w1_sb = wpool.tile([128, D], BF16, tag="w1")
            w2_sb = wpool.tile([K, D], BF16, tag="w2")
            ident = wpool.tile([128, 128], BF16, tag="ident")

            def emit_load(t, full):
                """Loads for tile t; big single DMAs unless group-granular."""
                st = spool.tile([128, NCH * TBMAX], s_dt, tag="st")
                bt = bpool.tile([128, NCH * TBMAX], BF16, tag="bt")
                n = NCH * TBS[t]
                if full:
                    nc.sync.dma_start(bt[:, :n], bT_d[:, offs[t] : offs[t + 1]])
                    nc.sync.dma_start(st[:, :n], sT_d[:, offs[t] : offs[t + 1]])
                else:
                    gn = n // G
                    for g in range(G):
                        sl = slice(g * gn, (g + 1) * gn)
                        dl = slice(offs[t] + g * gn, offs[t] + (g + 1) * gn)
                        nc.sync.dma_start(bt[:, sl], bT_d[:, dl])
                        nc.sync.dma_start(st[:, sl], sT_d[:, dl])
                return st, bt

            def emit_sub(t, st, bt):
                """diffT = sT - bT, group-granular so mm1 can chase loads."""
                n = NCH * TBS[t]
                if in_place_sub:
                    dt = st
                else:
                    dt = dpool.tile([128, NCH * TBMAX], BF16, tag="dt")
                gn = n // G
                for g in range(G):
                    sl = slice(g * gn, (g + 1) * gn)
                    if in_place_sub:
                        nc.vector.tensor_sub(dt[:, sl], st[:, sl], bt[:, sl])
                    else:
                        # fp8 -> bf16 on the (idle) activation engine keeps
                        # the DVE sub in its 2x 16-bit mode.
                        nc.scalar.copy(dt[:, sl], st[:, sl])
                        nc.vector.tensor_sub(dt[:, sl], dt[:, sl], bt[:, sl])
                return bt, dt

            def emit_w2_transposes():
                """w2 = W_sel.T from w1 via PE transposes during the DMA ramp
                (PE and scalar are idle then; saves 1 MiB of weight DMA)."""
                for r in range(4):
                    ptr = ptrpool.tile([128, 8 * 128], BF16, tag="ptr")
                    for q in range(8):
                        j = 8 * r + q
                        nc.tensor.transpose(
                            ptr[:, 128 * q : 128 * (q + 1)],
                            w1_sb[:, 128 * j : 128 * (j + 1)],
                            ident[:],
                        )
                    nc.scalar.copy(w2_sb[:, 1024 * r : 1024 * (r + 1)], ptr[:])

            def emit_compute(t, bt, dt):
                tb = TBS[t]
                pt = pTpool.tile([K, TBMAX], F32, tag="pt")
                for j in range(NCH):
                    nc.tensor.matmul(
                        pt[:, :tb],
                        w1_sb[:, 128 * j : 128 * (j + 1)],
                        dt[:, tb * j : tb * (j + 1)],
                        start=(j == 0),
                        stop=(j == NCH - 1),
                    )
                tt = ttpool.tile([K, TBMAX], BF16, tag="tt")
                nc.scalar.copy(tt[:, :tb], pt[:, :tb])

                for g in range(G):
                    ot = opool.tile([128, GCH * TBMAX], BF16, tag="ot")
                    for pg in range(GCH // PCH):
                        # PCH mm2 chunks into one psum tile, drained by a
                        # single DVE add (fewer DVE instructions).
                        p2 = p2pool.tile([128, PCH * TBMAX], F32, tag="p2")
                        for jj in range(PCH):
                            j = GCH * g + PCH * pg + jj
                            nc.tensor.matmul(
                                p2[:, tb * jj : tb * (jj + 1)],
                                w2_sb[:, 128 * j : 128 * (j + 1)],
                                tt[:, :tb],
                                start=True,
                                stop=True,
                            )
                        j0 = GCH * g + PCH * pg
                        nc.vector.tensor_add(
                            ot[:, tb * PCH * pg : tb * PCH * (pg + 1)],
                            bt[:, tb * j0 : tb * (j0 + PCH)],
                            p2[:, : tb * PCH],
                        )
                    n = GCH * tb
                    dl = slice(offs[t] + g * n, offs[t] + (g + 1) * n)
                    nc.sync.dma_start(out_d[:, dl], ot[:, :n])

            # Ramp: arm the largest loads first so the first-armed DMA
            # queues have enough bytes to stream while the rest arm.
            tiles = {}
            l1 = emit_load(1, full=True)
            nc.sync.dma_start(w1_sb[:], w1_d[:])
            tiles[0] = emit_sub(0, *emit_load(0, full=False))
            masks.make_identity(nc, ident[:])
            emit_w2_transposes()
            tiles[1] = emit_sub(1, *l1)
            emit_compute(0, *tiles[0])
            for t in range(2, NT):
                tiles[t] = emit_sub(t, *emit_load(t, full=True))
                emit_compute(t - 1, *tiles[t - 1])
            emit_compute(NT - 1, *tiles[NT - 1])

    nc.compile()
    return nc


_NC_CACHE = {}


def _get_nc(src_dtype="fp8"):
    if src_dtype not in _NC_CACHE:
        _NC_CACHE[src_dtype] = _build(src_dtype)
    return _NC_CACHE[src_dtype]


def _pack_xT(x16):
    """[8192, 4096] -> [cores, 128, NCH*BS] transposed chunk-major tiles."""
    out = np.empty((N_CORES, 128, NCH * BS), dtype=x16.dtype)
    for c in range(N_CORES):
        o = 0
        for tb in TBS:
            blk = x16[c * BS + o : c * BS + o + tb]         # [tb, D]
            v = blk.reshape(tb, NCH, 128).transpose(2, 1, 0)  # [128, NCH, tb]
            out[c, :, NCH * o : NCH * (o + tb)] = v.reshape(128, NCH * tb)
            o += tb
    return out


def make_in_maps(inputs, src_dtype="fp8"):
    base = np.asarray(inputs["base"], dtype=np.float32)
    source = np.asarray(inputs["source"], dtype=np.float32)
    subspaces = np.asarray(inputs["subspaces"])
    W = np.asarray(inputs["W"], dtype=np.float32)
    assert base.shape == (B_FULL, D) and source.shape == (B_FULL, D)

    sel = np.asarray(subspaces[0]).astype(np.int64)  # shared index set
    W_sel = np.ascontiguousarray(W[:, sel])          # [D, K] f32
    # chunk-major layout: w1[p, 128*j + kk] = W_sel[128*j + p, kk]
    w1 = np.ascontiguousarray(
        W_sel.reshape(NCH, 128, K).transpose(1, 0, 2).reshape(128, D)
    ).astype(ml_dtypes.bfloat16)

    s_np = ml_dtypes.bfloat16 if src_dtype == "bf16" else ml_dtypes.float8_e4m3
    sT = _pack_xT(source.astype(s_np))
    bT = _pack_xT(base.astype(ml_dtypes.bfloat16))

    in_maps = []
    for c in range(N_CORES):
        in_maps.append({"sT": sT[c], "bT": bT[c], "w1": w1})
    return in_maps


def unpack_out(res_list):
    """Per-core [128, NCH*BS] bf16 -> [8192, 4096] f32."""
    out = np.empty((B_FULL, D), dtype=ml_dtypes.bfloat16)
    for c, r in enumerate(res_list):
        o = 0
        blk_all = r["out"]
        for tb in TBS:
            v = blk_all[:, NCH * o : NCH * (o + tb)].reshape(128, NCH, tb)
            out[c * BS + o : c * BS + o + tb] = v.transpose(2, 1, 0).reshape(tb, D)
            o += tb
    return out.astype(np.float32)


def run(inputs, trace=False, src_dtype="fp8", **_ignored):
    nc = _get_nc(src_dtype)
    in_maps = make_in_maps(inputs, src_dtype)
    res = run_bass_kernel_spmd(nc, in_maps, list(range(N_CORES)), trace=trace)
    out = unpack_out(res.results)
    return out, res


def kernel(**inputs) -> np.ndarray:
    out, _ = run(inputs, trace=False)
    return out


# revision 16
# speedup vs baseline: 1.0848x; 1.0848x over previous
"""Trainium2 Bass kernel for nn_LowRankRotatedSpaceIntervention.

Reference computation (B=8192, D=4096, r=512, k=128):
    sel  = subspaces[0]                  # shared index set (fast path)
    diff = (source - base) @ W           # [B, r]
    out  = base + diff[:, sel] @ W[:, sel].T

Only the selected k=128 columns of W matter:
    out = base + ((source - base) @ W_sel) @ W_sel.T,  W_sel = W[:, sel]

The problem is HBM-bound (per-core DMA ceiling ~320 GB/s while busy, PE
needs only ~45us of matmul at the throttled pstate), so the kernel is
organized purely around minimizing HBM bytes and keeping the DMA engines
continuously busy:

  * base/source are packed on the host into a TRANSPOSED chunk-major
    16-bit layout so the device needs no transposes: the contraction dim
    (d) is already on partitions.  All FLOPs of the reference graph
    (sub, both matmuls, final add) stay on device; the host only does
    dtype conversion, layout packing and the W-column gather.
  * device I/O is bf16 (base, out) and fp8-e4m3 (source; its rounding
    error only enters through the rank-k correction, contributing
    ~3e-3 relative).  Measured end-to-end rel err: 5.9e-3 (budget 2e-2).
  * w2 = W_sel.T is derived on-device by PE-transposing w1 during the
    DMA ramp instead of loading it (saves 1 MiB of weight traffic).
  * batch is cut into tiles of [256,256,256,128,128] rows: big tiles in
    the steady state (DMA efficiency), small tiles at the end so the
    serial mm2->add->store drain of the last tile is short.
  * emission is software-pipelined: tile t+1's loads+subs are emitted
    before tile t's mm/add phase so the in-order DVE runs subs(t+1)
    ahead of adds(t); the largest loads are armed first so the DMA
    queues never run dry during the ramp.

Per-core dataflow (BS=1024 rows/core, NCH=32 d-chunks of 128):
    sT/bT dram [128, 32*BS]: [p, off_t + j*TB_t + b] = x[bat_t + b, 128j + p]
    per batch tile t:
      diffT = bf16(sT) - bT                 (scalar convert + DVE sub)
      T^T[k,TB]  = sum_j w1_j.T @ diffT_j   (32 matmuls, psum f32)
      tt = bf16(T^T)                        (scalar engine copy)
      per chunk j: corrT_j = w2_j.T @ tt    (matmul, 4-chunk psum tiles)
      outT_j = bT_j + corrT_j               (DVE add, 4 chunks at a time)
      store outT in groups of 8 chunks
"""

import numpy as np
import ml_dtypes

import concourse.bass as bass
import concourse.tile as tile
from concourse import bacc, masks, mybir
from concourse.bass_utils import run_bass_kernel_spmd

N_CORES = 8
B_FULL = 8192
D = 4096
K = 128
BS = B_FULL // N_CORES        # 1024 rows per core
TBS = (256, 256, 256, 128, 128)  # batch tile sizes (sum = BS)
NT = len(TBS)
NCH = D // 128                # 32 contraction / output chunks
GCH = 8                       # chunks per store group
G = NCH // GCH                # 4 store groups per tile
PCH = 4                       # mm2 chunks per psum tile / DVE add
TBMAX = max(TBS)

assert sum(TBS) == BS

F32 = mybir.dt.float32
BF16 = mybir.dt.bfloat16
FP8 = mybir.dt.float8e4


def _build(src_dtype="fp8"):
    nc = bacc.Bacc("TRN2", target_bir_lowering=False, debug=False)

    s_dt = BF16 if src_dtype == "bf16" else FP8
    sT_d = nc.dram_tensor("sT", [128, NCH * BS], s_dt, kind="ExternalInput").ap()
    bT_d = nc.dram_tensor("bT", [128, NCH * BS], BF16, kind="ExternalInput").ap()
    # w1: chunk-major W_sel: w1[p, 128*j + kk] = W_sel[128*j + p, kk]
    # (w2 = W_sel.T is derived on-device by PE-transposing w1)
    w1_d = nc.dram_tensor("w1", [128, D], BF16, kind="ExternalInput").ap()
    out_d = nc.dram_tensor("out", [128, NCH * BS], BF16, kind="ExternalOutput").ap()

    in_place_sub = s_dt == BF16
    offs = [NCH * sum(TBS[:t]) for t in range(NT + 1)]  # dram col offsets

    with tile.TileContext(nc) as tc:
        with (
            tc.tile_pool(name="wpool", bufs=1) as wpool,
            tc.tile_pool(name="spool", bufs=4) as spool,
            tc.tile_pool(name="bpool", bufs=4) as bpool,
            tc.tile_pool(name="dpool", bufs=2) as dpool,
            tc.tile_pool(name="ttpool", bufs=2) as ttpool,
            tc.tile_pool(name="opool", bufs=4) as opool,
            tc.tile_pool(name="pT", bufs=2, space="PSUM") as pTpool,
            tc.tile_pool(name="p2", bufs=2, space="PSUM") as p2pool,
            tc.tile_pool(name="ptr", bufs=2, space="PSUM") as ptrpool,
        ):
            w1_sb = wpool.tile([128, D], BF16, tag="w1")
            w2_sb = wpool.tile([K, D], BF16, tag="w2")
            ident = wpool.tile([128, 128], BF16, tag="ident")

            def emit_load(t, full):
                """Loads for tile t; big single DMAs unless group-granular."""
                st = spool.tile([128, NCH * TBMAX], s_dt, tag="st")
                bt = bpool.tile([128, NCH * TBMAX], BF16, tag="bt")
                n = NCH * TBS[t]
                if full:
                    nc.sync.dma_start(bt[:, :n], bT_d[:, offs[t] : offs[t + 1]])
                    nc.sync.dma_start(st[:, :n], sT_d[:, offs[t] : offs[t + 1]])
                else:
                    gn = n // G
                    for g in range(G):
                        sl = slice(g * gn, (g + 1) * gn)
                        dl = slice(offs[t] + g * gn, offs[t] + (g + 1) * gn)
                        nc.sync.dma_start(bt[:, sl], bT_d[:, dl])
                        nc.sync.dma_start(st[:, sl], sT_d[:, dl])
                return st, bt

            def emit_sub(t, st, bt):
                """diffT = sT - bT, group-granular so mm1 can chase loads."""
                n = NCH * TBS[t]
                if in_place_sub:
                    dt = st
                else:
                    dt = dpool.tile([128, NCH * TBMAX], BF16, tag="dt")
                gn = n // G
                for g in range(G):
                    sl = slice(g * gn, (g + 1) * gn)
                    if in_place_sub:
                        nc.vector.tensor_sub(dt[:, sl], st[:, sl], bt[:, sl])
                    else:
                        # fp8 -> bf16 on the (idle) activation engine keeps
                        # the DVE sub in its 2x 16-bit mode.
                        nc.scalar.copy(dt[:, sl], st[:, sl])
                        nc.vector.tensor_sub(dt[:, sl], dt[:, sl], bt[:, sl])
                return bt, dt

            def emit_w2_transposes(r):
                """w2 group r = W_sel.T from w1 via PE transposes, interleaved
                into compute(t0) so they hide behind mm1/mm2 (saves 1 MiB of
                weight DMA without delaying the compute-chain start)."""
                ptr = ptrpool.tile([128, 8 * 128], BF16, tag="ptr")
                for q in range(8):
                    j = 8 * r + q
                    nc.tensor.transpose(
                        ptr[:, 128 * q : 128 * (q + 1)],
                        w1_sb[:, 128 * j : 128 * (j + 1)],
                        ident[:],
                    )
                nc.scalar.copy(w2_sb[:, 1024 * r : 1024 * (r + 1)], ptr[:])

            def emit_compute(t, bt, dt):
                tb = TBS[t]
                pt = pTpool.tile([K, TBMAX], F32, tag="pt")
                for j in range(NCH):
                    nc.tensor.matmul(
                        pt[:, :tb],
                        w1_sb[:, 128 * j : 128 * (j + 1)],
                        dt[:, tb * j : tb * (j + 1)],
                        start=(j == 0),
                        stop=(j == NCH - 1),
                    )
                tt = ttpool.tile([K, TBMAX], BF16, tag="tt")
                nc.scalar.copy(tt[:, :tb], pt[:, :tb])

                for g in range(G):
                    if t == 0:
                        emit_w2_transposes(g)
                    ot = opool.tile([128, GCH * TBMAX], BF16, tag="ot")
                    for pg in range(GCH // PCH):
                        # PCH mm2 chunks into one psum tile, drained by a
                        # single DVE add (fewer DVE instructions).
                        p2 = p2pool.tile([128, PCH * TBMAX], F32, tag="p2")
                        for jj in range(PCH):
                            j = GCH * g + PCH * pg + jj
                            nc.tensor.matmul(
                                p2[:, tb * jj : tb * (jj + 1)],
                                w2_sb[:, 128 * j : 128 * (j + 1)],
                                tt[:, :tb],
                                start=True,
                                stop=True,
                            )
                        j0 = GCH * g + PCH * pg
                        nc.vector.tensor_add(
                            ot[:, tb * PCH * pg : tb * PCH * (pg + 1)],
                            bt[:, tb * j0 : tb * (j0 + PCH)],
                            p2[:, : tb * PCH],
                        )
                    n = GCH * tb
                    dl = slice(offs[t] + g * n, offs[t] + (g + 1) * n)
                    nc.sync.dma_start(out_d[:, dl], ot[:, :n])

            # Ramp: the whole run is compute-chain-bound after loads drain,
            # so tile 0's data and w1 go absolutely first (interleaved per
            # group) to start mm1(t0) as early as possible.
            st0 = spool.tile([128, NCH * TBMAX], s_dt, tag="st")
            bt0 = bpool.tile([128, NCH * TBMAX], BF16, tag="bt")
            gn0 = NCH * TBS[0] // G
            for g in range(G):
                sl = slice(g * gn0, (g + 1) * gn0)
                nc.sync.dma_start(bt0[:, sl], bT_d[:, sl])
                nc.sync.dma_start(st0[:, sl], sT_d[:, sl])
                wc = slice(GCH * 128 * g, GCH * 128 * (g + 1))
                nc.sync.dma_start(w1_sb[:, wc], w1_d[:, wc])
            masks.make_identity(nc, ident[:])
            tiles = {0: emit_sub(0, st0, bt0)}
            tiles[1] = emit_sub(1, *emit_load(1, full=True))
            emit_compute(0, *tiles[0])
            for t in range(2, NT):
                tiles[t] = emit_sub(t, *emit_load(t, full=True))
                emit_compute(t - 1, *tiles[t - 1])
            emit_compute(NT - 1, *tiles[NT - 1])

    nc.compile()
    return nc


_NC_CACHE = {}


def _get_nc(src_dtype="fp8"):
    if src_dtype not in _NC_CACHE:
        _NC_CACHE[src_dtype] = _build(src_dtype)
    return _NC_CACHE[src_dtype]


def _pack_xT(x16):
    """[8192, 4096] -> [cores, 128, NCH*BS] transposed chunk-major tiles."""
    out = np.empty((N_CORES, 128, NCH * BS), dtype=x16.dtype)
    for c in range(N_CORES):
        o = 0
        for tb in TBS:
            blk = x16[c * BS + o : c * BS + o + tb]         # [tb, D]
            v = blk.reshape(tb, NCH, 128).transpose(2, 1, 0)  # [128, NCH, tb]
            out[c, :, NCH * o : NCH * (o + tb)] = v.reshape(128, NCH * tb)
            o += tb
    return out


def make_in_maps(inputs, src_dtype="fp8"):
    base = np.asarray(inputs["base"], dtype=np.float32)
    source = np.asarray(inputs["source"], dtype=np.float32)
    subspaces = np.asarray(inputs["subspaces"])
    W = np.asarray(inputs["W"], dtype=np.float32)
    assert base.shape == (B_FULL, D) and source.shape == (B_FULL, D)

    sel = np.asarray(subspaces[0]).astype(np.int64)  # shared index set
    W_sel = np.ascontiguousarray(W[:, sel])          # [D, K] f32
    # chunk-major layout: w1[p, 128*j + kk] = W_sel[128*j + p, kk]
    w1 = np.ascontiguousarray(
        W_sel.reshape(NCH, 128, K).transpose(1, 0, 2).reshape(128, D)
    ).astype(ml_dtypes.bfloat16)

    s_np = ml_dtypes.bfloat16 if src_dtype == "bf16" else ml_dtypes.float8_e4m3
    sT = _pack_xT(source.astype(s_np))
    bT = _pack_xT(base.astype(ml_dtypes.bfloat16))

    in_maps = []
    for c in range(N_CORES):
        in_maps.append({"sT": sT[c], "bT": bT[c], "w1": w1})
    return in_maps


def unpack_out(res_list):
    """Per-core [128, NCH*BS] bf16 -> [8192, 4096] f32."""
    out = np.empty((B_FULL, D), dtype=ml_dtypes.bfloat16)
    for c, r in enumerate(res_list):
        o = 0
        blk_all = r["out"]
        for tb in TBS:
            v = blk_all[:, NCH * o : NCH * (o + tb)].reshape(128, NCH, tb)
            out[c * BS + o : c * BS + o + tb] = v.transpose(2, 1, 0).reshape(tb, D)
            o += tb
    return out.astype(np.float32)


def run(inputs, trace=False, src_dtype="fp8", **_ignored):
    nc = _get_nc(src_dtype)
    in_maps = make_in_maps(inputs, src_dtype)
    res = run_bass_kernel_spmd(nc, in_maps, list(range(N_CORES)), trace=trace)
    out = unpack_out(res.results)
    return out, res


def kernel(**inputs) -> np.ndarray:
    out, _ = run(inputs, trace=False)
    return out
